# revision 33
# baseline (speedup 1.0000x reference)
"""Trainium2 Bass kernel for the dense_cnn problem — v5 (paired wide ops).

Computation (per image, C=256, H=W=56):
  t1 = depthwise 5x5 conv(x, w1), pad 2
  t2 = depthwise (3,1) conv(x, w2), pad (1,0)
  t4 = w4 @ unfold(t2, K=7, dil 3, pad 9)   (1x1 mix, C*7 -> C)
  out = t1 * t4

Sharding: pure data parallel, 2 images per core across 8 cores.

Design (all data bf16 except PSUM accumulation and the final output):
  - x uploaded once as bf16 in a padded 60x60 layout; all engines read it.
  - t4: ~13 bf16 matmul chunks per 448-px tile into PSUM (PE).
  - t1's 25 taps split across engines (Pool gets none: GpSimd shares its
    SBUF port with the DVE on trn2, so concurrent Pool elementwise work
    serializes against the always-busy DVE on real hardware):
      * PE taps as diag matmuls accumulating exactly in PSUM; the diag
        matrices are built on-chip (identity x per-partition weight) so
        no ~1 MB weight upload sits in front of the first matmul,
      * Act taps (w*shift(x)) into bf16 temps, one 896-wide op per
        row-adjacent tile pair to amortize Act's fixed op overhead,
      * DVE tree-merges the Act temps (bf16 2x mode, pair-wide), adds
        its own taps (tensor_scalar 4x + tensor_tensor 2x), folds the
        PSUM partial and computes the gate out = t1 * t4 in fp32.
  - t2: bf16 tensor_scalar/tensor_tensor chain on DVE (4x/2x modes),
    written into a 74-row padded buffer so t4's unfold is a free SBUF
    offset.
"""

import os
import sys

sys.path.insert(0, "/opt/trn_rl_repo")

import numpy as np
import ml_dtypes

import concourse.bass as bass
import concourse.bacc as bacc
import concourse.mybir as mybir
from concourse.tile import TileContext
from concourse.bass_utils import run_bass_kernel_spmd

# problem constants
N, C, H, W = 16, 256, 56, 56
K, DIL, PAD = 7, 3, 9
HW = H * W                  # 3136
N_CORES = 8
IMGS = N // N_CORES         # 2 images per core
CH = C // 128               # 2 channel halves
PT = 448                    # pixel tile (8 rows of 56)
NPT = HW // PT              # 7 pixel tiles
ROWS_PER_PT = PT // W       # 8
X60 = 60                    # padded x layout: 60 rows x 60 cols
T2ROWS = H + 2 * PAD        # 74 rows in padded t2 buffer
T2LEN = T2ROWS * W          # 4144

# tap split across engines (scan order t = 5*dy + dx), per unit
# (n, oh): unit index u = n*2 + oh.  (pe, act, dve, pool) counts per unit;
# taps are assigned in scan order: PE first, then Act, DVE, Pool.
# Pool gets NO taps — GpSimd shares its SBUF port with the DVE, so any
# Pool tensor_tensor work running concurrently with the (always-busy)
# DVE serializes against it on real HW (the cost model does not model
# the shared port).
# Balance (from the real NTFF engine-active times): PE 14, Act 9,
# DVE 2, with Act/DVE ops running 896-wide over row-adjacent tile pairs
# to amortize the fixed per-op SBUF-access overheads.
UNIT_SPLITS = [
    (14, 9, 2, 0),
    (14, 9, 2, 0),
    (14, 9, 2, 0),
    (14, 9, 2, 0),
]
MAX_POOL = 0
MAX_PE = max(s[0] for s in UNIT_SPLITS)


def _unit_taps(u):
    p, a, d, g = UNIT_SPLITS[u]
    assert p + a + d + g == 25
    taps = list(range(25))
    return (taps[:p], taps[p:p + a], taps[p + a:p + a + d],
            taps[p + a + d:])

f32 = mybir.dt.float32
bf16 = mybir.dt.bfloat16
MULT = mybir.AluOpType.mult
ADD = mybir.AluOpType.add

LAST_EXEC_TIME_NS = None
LAST_TRACE_PATH = None

_CACHE = {}


def _build_nc(reps=1):
    """Build the per-core bass program (same NEFF for all 8 cores)."""
    nc = bacc.Bacc()

    npe = MAX_PE
    xp = nc.dram_tensor("xb60", [IMGS, CH, 128, X60 * X60], bf16,
                        kind="ExternalInput")
    w4tp = nc.dram_tensor("w4tp", [K * C, C], bf16, kind="ExternalInput")
    # 128x128 identity; the w1 diag matrices are built on-chip from it
    # (ident * per-partition tap weight), saving a ~0.9 MB upload that
    # sat directly in front of the first matmul.
    ident = nc.dram_tensor("ident", [128, 128], bf16, kind="ExternalInput")
    wsc = nc.dram_tensor("wsc", [C, 28], f32, kind="ExternalInput")
    out = nc.dram_tensor("out", [IMGS, C, H, W], f32, kind="ExternalOutput")

    with TileContext(nc) as tc:
        with (
            tc.tile_pool(name="persist", bufs=1) as pp,
            tc.tile_pool(name="t2tmp", bufs=2) as t2pool,
            tc.tile_pool(name="acttmp", bufs=3) as apool,
            tc.tile_pool(name="pooltmp", bufs=3) as gpool,
            tc.tile_pool(name="foldsb", bufs=3) as fpool,
            tc.tile_pool(name="outsb", bufs=2) as opool,
            tc.tile_pool(name="ps1", bufs=5, space="PSUM") as ps1,
            tc.tile_pool(name="ps4", bufs=3, space="PSUM") as ps4,
        ):
            # ---- persistent SBUF ----
            x60 = [[pp.tile([128, X60 * X60], bf16, tag=f"x60_{n}_{h}",
                            name=f"x60_{n}_{h}")
                    for h in range(CH)] for n in range(IMGS)]
            t2p = [[pp.tile([128, T2LEN], bf16, tag=f"t2p_{n}_{h}",
                            name=f"t2p_{n}_{h}")
                    for h in range(CH)] for n in range(IMGS)]
            w4sb = pp.tile([128, 2 * K * C], bf16, tag="w4sb")
            w1dsb = pp.tile([128, npe * CH * 128], bf16, tag="w1dsb")
            wssb = pp.tile([128, CH * 28], f32, tag="wssb")
            identsb = pp.tile([128, 128], bf16, tag="identsb")

            # ---- one-time init: zero the t2p pad rows (never rewritten;
            # memsets on the otherwise-idle Pool engine, no DMA involved) ----
            for n in range(IMGS):
                for h in range(CH):
                    nc.gpsimd.memset(t2p[n][h][:, 0:PAD * W], 0.0)
                    nc.gpsimd.memset(t2p[n][h][:, (PAD + H) * W:T2LEN], 0.0)

            # ---- per-image pipeline ----
            import contextlib
            loop_cm = (tc.For_i(0, reps, 1,
                                hint_engines=(mybir.EngineType.PE,
                                              mybir.EngineType.DVE,
                                              mybir.EngineType.SP,
                                              mybir.EngineType.Activation,
                                              mybir.EngineType.Pool))
                       if reps > 1 else contextlib.nullcontext())
            with loop_cm:
                # DMA issue is STAGED: an op's wait on the DMA completion
                # counter covers every DMA issued before the op was
                # emitted, so each stage issues only what the next batch
                # of compute needs, then that compute is emitted.
                npe_taps = UNIT_SPLITS[0][0]

                def emit_diags(oh):
                    # build the w1 diag matrices while x uploads:
                    # w1dsb[:, s*128:(s+1)*128] = ident * w1[ch, tap].
                    # oh0's matrices gate the very first tap group, so they
                    # go on the DVE (fastest per-op); oh1's are not needed
                    # until that unit starts ~40 us in, so they go on the
                    # Act engine, which is otherwise idle until the first
                    # x plane lands — keeping the DVE's startup backlog
                    # (which gates the PE via ps1-tile recycling) short.
                    eng = nc.vector.tensor_scalar_mul if oh == 0 else \
                        nc.scalar.mul
                    for i in range(npe_taps):
                        s = i * CH + oh
                        eng(w1dsb[:, s * 128:(s + 1) * 128], identsb[:],
                            wssb[:, oh * 28 + i:oh * 28 + i + 1])

                def emit_t2(n, h, r0, r1):
                    # t2 (3,1) depthwise on DVE, bf16 fast modes, rows
                    # [r0, r1). t2 row r = sum_j w2[j] * x[r+j-1]; x row
                    # r' lives at xb60 row r'+2, col c at c+2.
                    nr = r1 - r0
                    xv = x60[n][h][:].rearrange("p (r c) -> p r c", c=X60)
                    t2int = t2p[n][h][:, (PAD + r0) * W:(PAD + r1) * W]
                    tA = t2pool.tile([128, 2 * HW], bf16, tag="t2t")
                    a0 = tA[:, 0:nr * W]
                    a1 = tA[:, HW:HW + nr * W]
                    s0 = h * 28 + 25
                    nc.vector.tensor_scalar_mul(
                        a0.rearrange("p (r c) -> p r c", c=W),
                        xv[:, 1 + r0:1 + r1, 2:2 + W], wssb[:, s0:s0 + 1])
                    nc.vector.tensor_scalar_mul(
                        a1.rearrange("p (r c) -> p r c", c=W),
                        xv[:, 2 + r0:2 + r1, 2:2 + W],
                        wssb[:, s0 + 1:s0 + 2])
                    nc.vector.tensor_tensor(
                        out=t2int, in0=a0, in1=a1, op=ADD)
                    nc.vector.tensor_scalar_mul(
                        a0.rearrange("p (r c) -> p r c", c=W),
                        xv[:, 3 + r0:3 + r1, 2:2 + W],
                        wssb[:, s0 + 2:s0 + 3])
                    nc.vector.tensor_tensor(
                        out=t2int, in0=t2int, in1=a0, op=ADD)

                def emit_taps_for(n, oh, pt):
                    # t1 PE taps: diag matmuls into P1 (only need x + the
                    # on-chip diag matrices, so they run long before t4)
                    xv = x60[n][oh][:].rearrange("p (r c) -> p r c", c=X60)
                    r0 = pt * ROWS_PER_PT
                    pt1 = ps1.tile([128, PT], f32)
                    for i in range(npe_taps):
                        ty, tx = divmod(i, 5)
                        rhs = xv[:, r0 + ty:r0 + ty + ROWS_PER_PT,
                                 tx:tx + W]
                        nc.tensor.matmul(
                            pt1[:],
                            w1dsb[:, (i * CH + oh) * 128:
                                  (i * CH + oh + 1) * 128],
                            rhs,
                            start=(i == 0),
                            stop=(i == npe_taps - 1))
                    return pt1

                look = 4
                XA = 22 * X60   # first x chunk: rows 0..22 of image 0 h0
                # stage 1: what the first tap groups + Act pair 0 need
                nc.sync.dma_start(
                    out=wssb[:].rearrange("p (h s) -> p h s", h=CH),
                    in_=wsc[:].rearrange("(h p) s -> p h s", p=128))
                nc.sync.dma_start(out=identsb[:], in_=ident[:])
                nc.sync.dma_start(out=x60[0][0][:, 0:XA],
                                  in_=xp[0, 0][:, 0:XA])
                emit_diags(0)
                # only tiles 0-1 read rows covered by the first x chunk
                u0_p1q = [emit_taps_for(0, 0, pt) for pt in range(2)]

                # stage 2: rest of image 0 (t2 + the remaining tap groups)
                nc.sync.dma_start(out=x60[0][0][:, XA:],
                                  in_=xp[0, 0][:, XA:])
                nc.sync.dma_start(out=x60[0][1][:], in_=xp[0, 1])
                emit_diags(1)
                u0_p1q += [emit_taps_for(0, 0, pt) for pt in range(2, look)]
                emit_t2(0, 0, 0, 56)
                emit_t2(0, 1, 0, 28)
                emit_t2(0, 1, 28, 56)

                # stage 3: w4 just in time for the first t4 group, then
                # image 1 (consumed much later)
                nc.sync.dma_start(
                    out=w4sb[:].rearrange("p (q o) -> p q o", o=C),
                    in_=w4tp[:].rearrange("(q p) o -> p q o", p=128))
                for h in range(CH):
                    nc.sync.dma_start(out=x60[1][h][:], in_=xp[1, h])

                for n in range(IMGS):
                    if n > 0:
                        for h in range(CH):
                            emit_t2(n, h, 0, 56)

                    for oh in range(CH):
                        pe_taps, act_taps, dve_taps, pool_taps = \
                            _unit_taps(n * CH + oh)
                        xv = x60[n][oh][:].rearrange("p (r c) -> p r c", c=X60)
                        oplane = opool.tile([128, HW], f32)

                        def emit_acts(pt, nt):
                            # t1 Act taps over nt row-adjacent tiles in one
                            # op each (rows are contiguous within an image,
                            # so a tile pair is just a taller row window);
                            # the wider op amortizes Act's ~190 ns fixed
                            # SBUF-access overhead.
                            r0 = pt * ROWS_PER_PT
                            na = len(act_taps)
                            atmp = apool.tile([128, 9 * 2 * PT], bf16)
                            sz = nt * PT
                            for i, t in enumerate(act_taps):
                                ty, tx = divmod(t, 5)
                                nc.scalar.mul(
                                    atmp[:, i * sz:(i + 1) * sz]
                                    .rearrange("p (r c) -> p r c", c=W),
                                    xv[:, r0 + ty:
                                       r0 + ty + nt * ROWS_PER_PT,
                                       tx:tx + W],
                                    wssb[:, oh * 28 + t:oh * 28 + t + 1])
                            return atmp

                        def emit_merge(pt, nt, atmp):
                            # DVE: wide tree reduction of the 9 Act temps,
                            # + DVE's own taps, + per-tile fold of the PE
                            # partial and the gate.
                            r0 = pt * ROWS_PER_PT
                            sz = nt * PT
                            mrg = gpool.tile([128, 2 * PT], bf16)
                            nc.vector.tensor_tensor(
                                out=atmp[:, 0:4 * sz], in0=atmp[:, 0:4 * sz],
                                in1=atmp[:, 4 * sz:8 * sz], op=ADD)
                            nc.vector.tensor_tensor(
                                out=atmp[:, 0:2 * sz], in0=atmp[:, 0:2 * sz],
                                in1=atmp[:, 2 * sz:4 * sz], op=ADD)
                            nc.vector.tensor_tensor(
                                out=atmp[:, 0:sz], in0=atmp[:, 0:sz],
                                in1=atmp[:, sz:2 * sz], op=ADD)
                            nc.vector.tensor_tensor(
                                out=mrg[:, 0:sz], in0=atmp[:, 0:sz],
                                in1=atmp[:, 8 * sz:9 * sz], op=ADD)
                            for i, t in enumerate(dve_taps):
                                # bf16 fast-mode pair: tensor_scalar (4x)
                                # product + tensor_tensor (2x) add
                                ty, tx = divmod(t, 5)
                                dtmp = gpool.tile([128, 2 * PT], bf16)
                                nc.vector.tensor_scalar_mul(
                                    dtmp[:, 0:sz]
                                    .rearrange("p (r c) -> p r c", c=W),
                                    xv[:, r0 + ty:
                                       r0 + ty + nt * ROWS_PER_PT,
                                       tx:tx + W],
                                    wssb[:, oh * 28 + t:oh * 28 + t + 1])
                                nc.vector.tensor_tensor(
                                    out=mrg[:, 0:sz], in0=mrg[:, 0:sz],
                                    in1=dtmp[:, 0:sz], op=ADD)
                            for w_ in range(nt):
                                # fold exact PE partial (PSUM fp32) + gate
                                t1f = fpool.tile([128, PT], f32)
                                nc.vector.tensor_tensor(
                                    out=t1f[:],
                                    in0=mrg[:, w_ * PT:(w_ + 1) * PT],
                                    in1=p1q[pt + w_][:], op=ADD)
                                nc.vector.tensor_tensor(
                                    out=oplane[:, (pt + w_) * PT:
                                               (pt + w_ + 1) * PT],
                                    in0=t1f[:], in1=p4q[pt + w_][:],
                                    op=MULT)

                        # Tap groups run `look` tiles ahead of the t4
                        # groups: the in-order PE queue must not stall on a
                        # t4 while its t2/w4 inputs are still in flight.
                        if n == 0 and oh == 0:
                            p1q = u0_p1q
                        else:
                            p1q = [emit_taps_for(n, oh, pt)
                                   for pt in range(look)]
                        p4q = []
                        pend = {}

                        for pt in range(NPT):
                            r0 = pt * ROWS_PER_PT

                            # --- t4: bf16 matmul group (PE). For chunk
                            # (k, ch), output row r reads t2p padded row
                            # r0+r+3k: rows landing in the zero pad rows
                            # contribute nothing, so each chunk is narrowed
                            # to its valid row range (dropped entirely when
                            # empty). Full-coverage chunks go first so every
                            # PSUM element's first writer has start=True;
                            # ch0 before ch1 so the group can start before
                            # image-half 1's t2 is ready ---
                            pt4 = ps4.tile([128, PT], f32)
                            p4q.append(pt4)
                            qs = []
                            for q in range(K * CH):
                                k, ch = divmod(q, CH)
                                lo = max(0, PAD - r0 - k * DIL)
                                hi = min(ROWS_PER_PT, PAD + H - r0 - k * DIL)
                                if hi > lo:
                                    qs.append((q, lo, hi))
                            qs.sort(key=lambda t: (not (t[1] == 0 and
                                                        t[2] == ROWS_PER_PT),
                                                   t[0] % CH, t[0] // CH))
                            for j, (q, lo, hi) in enumerate(qs):
                                k, ch = divmod(q, CH)
                                row0 = r0 + k * DIL + lo
                                rhs = t2p[n][ch][:, row0 * W:
                                                 row0 * W + (hi - lo) * W]
                                nc.tensor.matmul(
                                    pt4[:, lo * W:hi * W],
                                    w4sb[:, q * C + oh * 128:
                                         q * C + oh * 128 + 128],
                                    rhs,
                                    start=(j == 0), stop=(j == len(qs) - 1))
                            if pt + look < NPT:
                                p1q.append(emit_taps_for(n, oh, pt + look))

                            # Act taps are emitted at the START of each tile
                            # pair, the DVE merge at its END (one tile of
                            # slack for the Act engine to fill the temps).
                            # The final singleton tile's temps are emitted
                            # one tile early for the same slack — otherwise
                            # the unit's tail serializes behind ~5 us of
                            # Act work.
                            if pt % 2 == 0 and pt < NPT - 1:
                                pend[pt] = (2, emit_acts(pt, 2))
                            if pt == NPT - 2:
                                pend[NPT - 1] = (1, emit_acts(NPT - 1, 1))
                            if pt % 2 == 1 or pt == NPT - 1:
                                base = pt - (pt % 2)
                                nt, atmp = pend.pop(base)
                                emit_merge(base, nt, atmp)

                            # output DMAs per ~quarter plane, each issued
                            # as soon as its gates are done (short tail)
                            oflat = out[n, oh * 128:(oh + 1) * 128] \
                                .rearrange("p r c -> p (r c)")
                            odma = {1: (0, 2 * PT), 3: (2 * PT, 4 * PT),
                                    5: (4 * PT, 6 * PT), 6: (6 * PT, HW)}
                            if pt in odma:
                                lo, hi = odma[pt]
                                nc.sync.dma_start(out=oflat[:, lo:hi],
                                                  in_=oplane[:, lo:hi])

    nc.compile()
    return nc


def _prep_inputs(x, w1, w2, w4):
    """Host-side layout prep shared by all cores (weights) + per-core x."""
    x = np.ascontiguousarray(np.asarray(x, dtype=np.float32))
    w1 = np.asarray(w1, dtype=np.float32).reshape(C, 5, 5)
    w2 = np.asarray(w2, dtype=np.float32).reshape(C, 3)
    w4 = np.ascontiguousarray(np.asarray(w4, dtype=np.float32))

    npe = MAX_PE
    # w4 [C, C*K] -> [(k, c), o], bf16
    w4tp = np.ascontiguousarray(
        w4.reshape(C, C, K).transpose(2, 1, 0).reshape(K * C, C)
    ).astype(ml_dtypes.bfloat16)

    ident = np.eye(128, dtype=np.float32).astype(ml_dtypes.bfloat16)

    wsc = np.ascontiguousarray(
        np.concatenate([w1.reshape(C, 25), w2], axis=1))

    # padded per-core x: [IMGS, CH, 128, 60*60] bf16
    xp_all = np.zeros((N, CH, 128, X60, X60), dtype=np.float32)
    xr = x.reshape(N, CH, 128, H, W)
    xp_all[:, :, :, 2:2 + H, 2:2 + W] = xr
    xp_all = xp_all.reshape(N, CH, 128, X60 * X60).astype(ml_dtypes.bfloat16)

    shared = {"w4tp": w4tp, "ident": ident, "wsc": wsc}
    in_maps = []
    for c in range(N_CORES):
        m = dict(shared)
        m["xb60"] = np.ascontiguousarray(xp_all[c * IMGS:(c + 1) * IMGS])
        in_maps.append(m)
    return in_maps


def kernel(x, w1, w2, w4):
    global LAST_EXEC_TIME_NS, LAST_TRACE_PATH
    if "nc" not in _CACHE:
        _CACHE["nc"] = _build_nc()
    nc = _CACHE["nc"]

    in_maps = _prep_inputs(x, w1, w2, w4)
    trace = os.environ.get("BASS_KERNEL_TRACE", "0") == "1"
    try:
        res = run_bass_kernel_spmd(nc, in_maps, core_ids=list(range(N_CORES)),
                                   trace=trace)
    except ModuleNotFoundError:
        res = run_bass_kernel_spmd(nc, in_maps, core_ids=list(range(N_CORES)),
                                   trace=False)
    LAST_EXEC_TIME_NS = res.exec_time_ns
    if res.instructions_and_trace is not None:
        LAST_TRACE_PATH = res.instructions_and_trace[1]
    out = np.concatenate([r["out"] for r in res.results], axis=0)
    return out.astype(np.float32)

